# revision 29
# baseline (speedup 1.0000x reference)
"""Bonsai soft-decision-tree forward pass on 8 TRN2 NeuronCores.

Math (per batch row b):
    pp    = (x @ Z) / 64                      [B, 64]
    tb_j  = tanh(4 * pp @ T[j])               internal nodes j = 0..14
    I_0   = 1;  I_{2j+1} = I_j * 0.5*(1+tb_j);  I_{2j+2} = I_j * 0.5*(1-tb_j)
    score = sum_n I_n * (pp @ W[n]) * tanh(4 * pp @ V[n])   [B, 32]

Distribution: pure data parallel. The batch (100000 rows) is padded to
8 * 12544 and split across the 8 cores; Z/T/V/W are replicated.

Per-core layouts (prepared on host, all matmul contractions end up on
SBUF partitions):
    xT   [128, 4, Bc]  xT[p, c, b] = x[b, 128*c + p]
    Zs   [128, 4, 64]  Zs[p, c, d] = Z[128*c + p, d] / 64
    T4T  [64, 16]      (4*T).T zero-padded from 15 to 16 columns
    Wcm  [64, 1024]    W.transpose(1,2,0) (channel-major, node padded to 32)
    Vcm  [64, 1024]    (4*V).transpose(1,2,0), same packing

Default mode "sd" streams 128-row batch tiles through a 4-engine
pipeline balanced so the vector and scalar engines (the co-bottlenecks
at ~88% busy) each stay near their throughput floor:
    PE    pp (PSUM-bank-shared with the tree logits), wpA/wpB/vp matmuls
    ACT   tanh(4*vp) [992 elems/tile], the pp pair PSUM->SBUF copies,
          and the indicator sigmoids
    DVE   wpI = wp*I for channels 0:16 (the unavoidable 1x fp32-PSUM
          drain), then a custom fused multiply + per-channel-reset
          prefix-sum op (BONSAI_MUL_SCAN_SD, 2x_1p on packed bf16 SBUF)
          that forms sum_n wpI*tv per channel in one pass
    Pool  wpI for channels 16:32 via scalar_tensor_tensor, plus the
          strided score extracts (one per 8 tiles) and I_g memsets
PSUM (8 banks): pp+tb share one bank; wpA double-buffered (its ring is
the DVE chain), wpB single (gpsimd drains it fast enough), vp
double-buffered (decouples tanh(t) from the tile t+1 matmuls).
Indicators per group: one sigmoid pair + a level-batched product
recursion on DVE.  Scores are written bf16 and upcast on the host.
"""

import numpy as np

# Problem constants (fixed by the reference).
INT_N = 15
TOT_N = 31
NF = 512
PD = 64
NCH = 32
BATCH = 100000
SIG = 4.0

N_CORES = 8
P = 128                       # partition count / batch tile rows
NODES = 32                    # node dim padded (node 31 is zero weights)
FREE = NODES * NCH            # 1024: elementwise width per batch row
B_RAW = BATCH // N_CORES      # 12500
TILES_FULL = (B_RAW + P - 1) // P   # 98
B_CORE = TILES_FULL * P       # 12544

_CACHE = {}


def _flat(ap3):
    """[p, c, n] tile view -> [p, c*n]."""
    return ap3.rearrange("p c n -> p (c n)")


def _ensure_scan_op():
    """Register the fused multiply + running-sum custom DVE op.

    out[p, k] = sum_{j<=k} in0[p, j] * in1[p, j]  (fp32 accumulate).
    One 1-elem/cycle pass does the wp (PSUM) multiply and the node
    reduction together; per-channel sums are recovered from the running
    sum at the node-block boundaries.
    """
    import concourse.dve_ops as dve_ops
    from concourse.dve_spec import AluOp, Spec, Src0, Src1, _has_src1, lower, scan
    from concourse.dve_uop import DveOpSpec

    name = "BONSAI_MUL_SCAN"
    for op in dve_ops.OPS:
        if op.name == name:
            return op

    def _ref(in0, in1, s0, s1, imm2):
        prod = in0.astype(np.float32) * np.asarray(in1).astype(np.float32)
        # the hardware scan runs over the flat stream order, crossing any
        # inner AP-dim boundaries
        flat = prod.reshape(prod.shape[0], -1)
        rs = np.cumsum(flat, axis=-1, dtype=np.float32)
        return rs.reshape(prod.shape)

    spec = Spec(body=scan(AluOp.ADD, Src0 * Src1), reference=_ref)
    row = max(dve_ops._SUB_OPCODE_FOR_NAME.values()) + 1
    assert row < 0x20
    dve_ops._SUB_OPCODE_FOR_NAME[name] = row
    shas = {}
    for ver in ("v3", "v4"):
        s = DveOpSpec(name=name, opcode=row, uops=lower(spec, ver=ver),
                      rd1_en=_has_src1(spec))
        shas[ver] = s.sha(ver)
    op = dve_ops.DveOp(name, spec, subdim=False, uops_sha=shas)
    dve_ops.OPS.append(op)
    dve_ops.CUSTOM_DVE_SPECS[name] = spec
    return op


def _ensure_scan_sd_op():
    """Register the per-channel-reset fused multiply + prefix-sum DVE op.

    out[p, c, k] = sum_{j<=k} in0[p, c, j] * in1[p, c, j]  (fp32 accumulate,
    restarting at every subdim row c).  SUB_DIM_DONE resets the running sum at
    [*, S, N] row boundaries, so each channel's sum never mixes with its
    neighbours and a bf16 output stays accurate.  The carry chain packs two
    elements per cycle (mul pair + temporal-carry adds across two ALU slices
    with the NEXT_ALU back-edge), so the op supports the 2x_1p perf mode.
    """
    import concourse.dve_ops as dve_ops
    from concourse.dve_spec import AluOp, Spec, Src0, Src1, _has_src1, lower, scan
    from concourse.dve_uop import DveOpSpec

    name = "BONSAI_MUL_SCAN_SD"
    for op in dve_ops.OPS:
        if op.name == name:
            return op

    def _ref(in0, in1, s0, s1, imm2):
        prod = in0.astype(np.float32) * np.asarray(in1).astype(np.float32)
        return np.cumsum(prod, axis=-1, dtype=np.float32)

    spec = Spec(body=scan(AluOp.ADD, Src0 * Src1), reference=_ref)
    row = max(dve_ops._SUB_OPCODE_FOR_NAME.values()) + 1
    assert row < 0x20
    dve_ops._SUB_OPCODE_FOR_NAME[name] = row
    shas = {}
    for ver in ("v3", "v4"):
        s = DveOpSpec(name=name, opcode=row, uops=lower(spec, ver=ver),
                      rd1_en=_has_src1(spec))
        shas[ver] = s.sha(ver)
    op = dve_ops.DveOp(name, spec, subdim=True, uops_sha=shas)
    dve_ops.OPS.append(op)
    dve_ops.CUSTOM_DVE_SPECS[name] = spec
    return op


def _build(tiles, group, fp32=True, mode=None):
    """Build the single-core Bass/Tile program for `tiles` 128-row tiles.

    mode: "fp32" | "bf16" | "scan".  "scan" uses the fused multiply +
    running-sum custom DVE op; the score DMA'd out is then the per-row
    cumulative channel sum and the host takes the channel diff.
    """
    from contextlib import ExitStack

    import concourse.bacc as bacc
    import concourse.bass as bass
    import concourse.mybir as mybir
    import concourse.tile as tile

    if mode is None:
        mode = "fp32" if fp32 else "bf16"
    fp32 = mode == "fp32"
    scan_op = _ensure_scan_op() if mode in ("scan", "scan2") else None
    sd_op = _ensure_scan_sd_op() if mode in ("sd", "fs") else None

    f32 = mybir.dt.float32
    dt = f32 if fp32 else mybir.dt.bfloat16
    b_core = tiles * P

    nc = bacc.Bacc("TRN2", target_bir_lowering=False, debug=False)

    xT = nc.dram_tensor("xT", [P, 4, b_core], dt, kind="ExternalInput")
    Zs = nc.dram_tensor("Zs", [P, 4, PD], dt, kind="ExternalInput")
    t4w = 32 if mode in ("sd", "fs") else 16
    T4T = nc.dram_tensor("T4T", [PD, t4w], dt, kind="ExternalInput")
    Wcm = nc.dram_tensor("Wcm", [PD, FREE], dt, kind="ExternalInput")
    Vcm = nc.dram_tensor("Vcm", [PD, FREE], dt, kind="ExternalInput")
    score_dt = dt if mode in ("sd", "fs") else f32
    score_d = nc.dram_tensor("score", [b_core, NCH], score_dt,
                             kind="ExternalOutput")

    mult = mybir.AluOpType.mult
    Sigmoid = mybir.ActivationFunctionType.Sigmoid
    Tanh = mybir.ActivationFunctionType.Tanh

    import json
    import os
    cfg = json.loads(os.environ.get("BONSAI_CFG", "{}"))

    # ramp: small leading groups so the vector engine (the bottleneck)
    # starts its phase-3 work as early as possible instead of waiting
    # for a full first group's loads + projections; fs adds a tail ramp so
    # the final stage-B drain (scans with no stage-A work left to overlap)
    # is short
    default_ramp = (6, 7) if mode in ("sd", "fs") else (2, 4, 6, 7)
    head = [g for g in cfg.get("ramp", default_ramp) if g < group]
    tail = [g for g in cfg.get("tramp", (5, 4, 2, 2) if mode == "fs" else ())
            if g <= group]
    body = tiles - sum(head) - sum(tail)
    if body < 0:
        head, tail, body = [], [], tiles
    sizes = head + [group] * (body // group)
    if body % group:
        sizes.append(body % group)
    sizes += tail
    groups = []
    t0 = 0
    for g in sizes:
        if mode == "scan2":
            assert g % 2 == 0, "scan2 needs even group sizes"
        groups.append((t0, g))
        t0 += g
    assert t0 == tiles

    xbufs = cfg.get("xbufs", 8)
    vbufs = cfg.get("vbufs", 20)
    ppb = cfg.get("ppb", 2)
    tbb = cfg.get("tbb", 2)
    wvb = cfg.get("wvb", 1)

    with tile.TileContext(nc) as tc, ExitStack() as ctx:
        cpool = ctx.enter_context(tc.tile_pool(name="consts", bufs=1))
        xpool = ctx.enter_context(tc.tile_pool(name="xin", bufs=xbufs))
        ppool = ctx.enter_context(
            tc.tile_pool(name="ppt",
                         bufs=group + cfg.get("ppx", 12 if mode == "fs" else 4)))
        gpool = ctx.enter_context(
            tc.tile_pool(name="grp", bufs=cfg.get("gbufs", 3)))
        vpool = ctx.enter_context(tc.tile_pool(name="elem", bufs=vbufs))
        spool = ctx.enter_context(
            tc.tile_pool(name="score", bufs=cfg.get("sbufs", 4)))
        if mode == "scan2":
            ppb = tbb = wvb = 1
        if mode in ("sd", "fs"):
            # sd: 8 PSUM banks: pp-pair 1 + tb 1 + wpA 1 + wpB 1 + vp 2x2
            # fs: 7 banks: pp+tb 1 + wp 1x2 + vp 2x2
            ppb = cfg.get("ppb_sd", 1)
            tbb = cfg.get("tbb_sd", 1)
            wvb = cfg.get("wvb_sd", 2)
        wpb = cfg.get("wpb", 1)
        if mode == "scan" and wpb > 1:
            # wp double-buffered so the next tile's wp matmuls overlap the
            # current scan; costs pp/tb their double buffers (PSUM = 8 banks)
            ppb = tbb = 1
        ps_pp = ctx.enter_context(
            tc.tile_pool(name="ps_pp", bufs=ppb, space=bass.MemorySpace.PSUM))
        ps_tb = ctx.enter_context(
            tc.tile_pool(name="ps_tb", bufs=tbb, space=bass.MemorySpace.PSUM))
        ps_wv = ctx.enter_context(
            tc.tile_pool(name="ps_wv", bufs=wvb, space=bass.MemorySpace.PSUM))
        if mode in ("sd", "fs"):
            ps_wp = ctx.enter_context(tc.tile_pool(
                name="ps_wp", bufs=cfg.get("wpb_sd", 1),
                space=bass.MemorySpace.PSUM))
        else:
            ps_wp = (ctx.enter_context(
                tc.tile_pool(name="ps_wp", bufs=wpb, space=bass.MemorySpace.PSUM))
                if mode == "scan" and wpb > 1 else ps_wv)
        ps_w2 = (ctx.enter_context(
            tc.tile_pool(name="ps_w2", bufs=1, space=bass.MemorySpace.PSUM))
            if mode == "scan2" else None)

        Zs_sb = cpool.tile([P, 4, PD], dt)
        # constants replicated into partitions 64:128 in sd/fs modes so the
        # odd pp pairs (whose ppt lives on the upper partition half) can
        # feed matmuls directly (lhsT/rhs must share a base partition)
        cheight = P if mode in ("sd", "fs") else PD
        T4T_sb = cpool.tile([cheight, t4w], dt)
        Wcm_sb = cpool.tile([cheight, FREE], dt)
        Vcm_sb = cpool.tile([cheight, FREE], dt)
        zt_loaded = False
        wv_loaded = False

        # dummy activations at t~0 so the ACT spline-table loads (~1.3us
        # each) overlap the DMA-bound startup instead of stalling the
        # first real sigmoid/tanh mid-pipeline
        warm = cpool.tile([1, 2], f32)
        nc.gpsimd.memset(warm[:], 0.0)
        nc.scalar.activation(warm[:, 0:1], warm[:, 0:1], Sigmoid)
        nc.scalar.activation(warm[:, 1:2], warm[:, 1:2], Tanh)
        if mode in ("sd", "fs") and cfg.get("pe_warm", 10):
            # dummy matmuls keep PE continuously busy from t=0 so the
            # p-state ramp (low->mid->full over ~3us) completes before the
            # first x tile lands; they have no data deps
            wsrc = cpool.tile([PD, P], dt)
            nc.gpsimd.memset(wsrc[:], 0.0)
            if mode == "fs":
                wdst3 = ps_wv.tile([P, NCH, NODES], f32, tag="wv",
                                   bufs=cfg.get("wvb_fs", 3))
                wdst = wdst3[:].rearrange("p c n -> p (c n)")[:, 0:P]
            else:
                wdst = ps_wp.tile(
                    [P, P], f32, tag="wpA",
                    bufs=1 if cfg.get("c8", 0) else cfg.get("wpbA", 2))[:]
            for _ in range(cfg.get("pe_warm", 10)):
                nc.tensor.matmul(wdst, wsrc[:], wsrc[:],
                                 start=True, stop=True)

        sd_pending = []

        def emit_sd_phase3(g0, gsz, ppts, I_g):
            okt = cfg.get("okt", 8)      # batch tiles per extract + DMA
            ext_eng = cfg.get("ext_eng", "dve")
            HC = NCH // 2
            lag = []
            rs4_box = [None]
            QC = NCH // 4           # 8-channel quarter
            c8 = cfg.get("c8", 0)   # 8/16/8 DVE/gpsimd channel split
            for ti in range(gsz):
                ppt = ppts[ti]
                vp_ps = ps_wv.tile([P, NCH, NODES], f32, tag="vp")
                fv = _flat(vp_ps[:])
                if c8:
                    # wp split 8/16/8: DVE drains wpa (bank-packed
                    # half-pair alternating by tile parity), gpsimd stt
                    # ops drain wpb1 and the wpb2 half-pair
                    if ti % 2 == 0:
                        wpa2 = ps_wp.tile([P, 2, QC, NODES], f32, tag="wpA")
                        wpb2p = ps_wp.tile([P, 2, QC, NODES], f32,
                                           tag="wpB2")
                    par = ti % 2
                    wpa = wpa2[:, par]
                    wpb2 = wpb2p[:, par]
                    wpb1 = ps_wp.tile([P, 2 * QC, NODES], f32, tag="wpB1")
                    nc.tensor.matmul(wpa.rearrange("p c n -> p (c n)"), ppt,
                                     Wcm_sb[:, 0:256], start=True, stop=True)
                    nc.tensor.matmul(
                        wpb1[:].rearrange("p c n -> p (c n)"), ppt,
                        Wcm_sb[:, 256:768], start=True, stop=True)
                    nc.tensor.matmul(wpb2.rearrange("p c n -> p (c n)"), ppt,
                                     Wcm_sb[:, 768:1024], start=True,
                                     stop=True)
                else:
                    # 16/16 split: wpA feeds the DVE multiply (ring 2),
                    # wpB the gpsimd stt (ring 1).  ppt may live on either
                    # partition half; use the matching const replica
                    pbp = ppt.base_partition()
                    Wr = Wcm_sb[pbp:pbp + PD]
                    Vr = Vcm_sb[pbp:pbp + PD]
                    wpa3 = ps_wp.tile([P, HC, NODES], f32, tag="wpA",
                                      bufs=cfg.get("wpbA", 2))
                    wpb3 = ps_wp.tile([P, HC, NODES], f32, tag="wpB",
                                      bufs=cfg.get("wpbB", 1))
                    wpa = wpa3[:]
                    nc.tensor.matmul(_flat(wpa3[:]), ppt, Wr[:, 0:512],
                                     start=True, stop=True)
                    nc.tensor.matmul(_flat(wpb3[:]), ppt,
                                     Wr[:, 512:1024], start=True,
                                     stop=True)
                nc.tensor.matmul(fv[:, 0:512], ppt, Vr[:, 0:512],
                                 start=True, stop=True)
                nc.tensor.matmul(fv[:, 512:1024], ppt, Vr[:, 512:1024],
                                 start=True, stop=True)
                spair = cfg.get("spair", 1)
                kk = ti % spair if spair > 1 else 0
                if kk == 0:
                    tv2 = vpool.tile([P, spair, NCH, TOT_N], dt, tag="tv")
                    wpi2 = vpool.tile([P, spair, NCH, TOT_N], dt, tag="wpi")
                tv = tv2[:, kk]
                wpi = wpi2[:, kk]
                nc.scalar.activation(tv, vp_ps[:, :, :TOT_N], Tanh)
                # wpI = wp * I (downcast to bf16 SBUF)
                ib = I_g[:, ti, :]
                ibq = bass.AP(ib.tensor, ib.offset,
                              [ib.ap[0], [0, QC], [ib.ap[1][0], TOT_N]])
                ibh = bass.AP(ib.tensor, ib.offset,
                              [ib.ap[0], [0, 2 * QC], [ib.ap[1][0], TOT_N]])
                if c8:
                    nc.vector.tensor_mul(wpi[:, :QC, :], wpa[:, :, :TOT_N],
                                         ibq)
                    nc.gpsimd.scalar_tensor_tensor(
                        wpi[:, QC:3 * QC, :], wpb1[:, :, :TOT_N], 1.0, ibh,
                        op0=mult, op1=mult)
                    nc.gpsimd.scalar_tensor_tensor(
                        wpi[:, 3 * QC:, :], wpb2[:, :, :TOT_N], 1.0, ibq,
                        op0=mult, op1=mult)
                else:
                    # DVE drains channels 0:ncd of wpA; gpsimd picks up the
                    # rest of wpA (small second stt) plus all of wpB.  For
                    # the final tiles the split shifts toward gpsimd so the
                    # vector engine (which runs ~2 tiles behind the scalar
                    # engine) drains the pipeline sooner
                    ncd = cfg.get("ncd", 13)
                    if g0 + ti >= tiles - cfg.get("tail_n", 3):
                        ncd = cfg.get("tail_ncd", 10)
                    ibd = bass.AP(ib.tensor, ib.offset,
                                  [ib.ap[0], [0, ncd], [ib.ap[1][0], TOT_N]])
                    nc.vector.tensor_mul(wpi[:, :ncd, :],
                                         wpa[:, :ncd, :TOT_N], ibd)
                    # big wpB stt first: its end releases the wpB ring
                    nc.gpsimd.scalar_tensor_tensor(
                        wpi[:, HC:, :], wpb3[:, :, :TOT_N], 1.0, ibh,
                        op0=mult, op1=mult)
                    if ncd < HC:
                        iba = bass.AP(
                            ib.tensor, ib.offset,
                            [ib.ap[0], [0, HC - ncd], [ib.ap[1][0], TOT_N]])
                        nc.gpsimd.scalar_tensor_tensor(
                            wpi[:, ncd:HC, :], wpa[:, ncd:, :TOT_N], 1.0,
                            iba, op0=mult, op1=mult)
                # scan/extract for tile ti are deferred one tile (emitted
                # from the NEXT iteration) so back-to-back DVE ops are
                # never data-dependent and the gpsimd half gets slack
                if kk == spair - 1 or ti == gsz - 1:
                    lag.append((ti, wpi2, tv2, kk))
                while len(lag) > (1 if ti < gsz - 1 else 0):
                    lti, lwpi2, ltv2, lkk = lag.pop(0)
                    k = lti % okt
                    npr = lkk + 1
                    if k - lkk == 0:
                        rs4_new = vpool.tile([P, okt, NCH, TOT_N], dt,
                                             tag="rs4",
                                             bufs=cfg.get("rsb", 4))
                        rs4_box[0] = rs4_new
                    rs4 = rs4_box[0]
                    ro = rs4[:, k - lkk:k + 1].rearrange(
                        "p a c n -> p (a c) n")
                    # fused multiply + per-channel prefix sum at 2x (all
                    # operands packed bf16 in SBUF)
                    bi = nc.vector._custom_dve(
                        sd_op,
                        out=ro,
                        in0=lwpi2[:, :npr].rearrange("p a c n -> p (a c) n"),
                        in1=ltv2[:, :npr].rearrange("p a c n -> p (a c) n"))
                    bi.ins.perf_max = 1
                    if k == okt - 1 or lti == gsz - 1:
                        # one strided extract + one DMA per okt tiles
                        nk = k + 1
                        t0 = g0 + lti - k
                        last = rs4[:, :nk, :, TOT_N - 1]
                        od = score_d.ap()[t0 * P:(t0 + nk) * P, :]
                        if ext_eng == "dma":
                            r = rs4[:]
                            src = bass.AP(
                                r.tensor, r.offset + TOT_N - 1,
                                [r.ap[0], [NCH * TOT_N, nk], [TOT_N, NCH]])
                            nc.sync.dma_start(
                                od.rearrange("(k p) c -> p k c", k=nk), src)
                        else:
                            sck = spool.tile([P, okt, NCH], dt, tag="sck")
                            if ext_eng == "act":
                                nc.scalar.copy(sck[:, :nk, :], last)
                            elif ext_eng == "pool":
                                nc.gpsimd.tensor_copy(sck[:, :nk, :], last)
                            else:
                                nc.vector.tensor_copy(sck[:, :nk, :], last)
                            nc.sync.dma_start(
                                od.rearrange("(k p) c -> p k c", k=nk),
                                sck[:, :nk, :])

        # fs-mode state: stage-B work items (wp matmuls + fused scan +
        # score extract) queue up here and are emitted a fixed number of
        # tiles behind stage A (vp/tanh/tvi), continuously across group
        # seams.  vp and wp PSUM tiles share ONE 3-slot ring (tag "wv",
        # 6 banks): each slot alternates between a fast-drained vp (tanh,
        # ~1 period) and a slower-drained wp (fused scan, ~1.5 periods),
        # so neither PE refill ever head-blocks.
        fs_bq = []
        fs_bstate = {"rs8": None}

        def emit_fs_stageA(ti, ppts, I_g):
            tvx = cfg.get("tvx", 4)
            ppt = ppts[ti]
            pbp = ppt.base_partition()
            Vr = Vcm_sb[pbp:pbp + PD]
            vp_ps = ps_wv.tile([P, NCH, NODES], f32, tag="wv",
                               bufs=cfg.get("wvb_fs", 3))
            fv = _flat(vp_ps[:])
            nc.tensor.matmul(fv[:, 0:512], ppt, Vr[:, 0:512],
                             start=True, stop=True)
            nc.tensor.matmul(fv[:, 512:1024], ppt, Vr[:, 512:1024],
                             start=True, stop=True)
            tv = vpool.tile([P, NCH, TOT_N], dt, tag="tv",
                            bufs=cfg.get("tvb", 4))
            nc.scalar.activation(tv[:], vp_ps[:, :, :TOT_N], Tanh)
            tvi = vpool.tile([P, NCH, TOT_N], dt, tag="tvi",
                             bufs=cfg.get("tvib", 14))
            ib = I_g[:, ti, :]
            if tvx > 0:
                ibd = bass.AP(ib.tensor, ib.offset,
                              [ib.ap[0], [0, tvx], [ib.ap[1][0], TOT_N]])
                nc.vector.tensor_mul(tvi[:, :tvx], tv[:, :tvx, :], ibd)
            if tvx < NCH:
                ibp = bass.AP(
                    ib.tensor, ib.offset,
                    [ib.ap[0], [0, NCH - tvx], [ib.ap[1][0], TOT_N]])
                nc.gpsimd.scalar_tensor_tensor(
                    tvi[:, tvx:], tv[:, tvx:, :], 1.0, ibp,
                    op0=mult, op1=mult)
            return tvi

        def emit_fs_stageB(ti, g0, gsz, ppts, tvis):
            okt = cfg.get("okt", 8)
            ext_eng = cfg.get("fs_ext", "act")
            ppt = ppts[ti]
            pbp = ppt.base_partition()
            Wr = Wcm_sb[pbp:pbp + PD]
            wp_ps = ps_wv.tile([P, NCH, NODES], f32, tag="wv",
                               bufs=cfg.get("wvb_fs", 3))
            fw = _flat(wp_ps[:])
            nc.tensor.matmul(fw[:, 0:512], ppt, Wr[:, 0:512],
                             start=True, stop=True)
            nc.tensor.matmul(fw[:, 512:1024], ppt, Wr[:, 512:1024],
                             start=True, stop=True)
            k = ti % okt
            if k == 0:
                rs8 = spool.tile([P, okt, NCH, TOT_N], dt, tag="rs8",
                                 name="rs8", bufs=cfg.get("rsb", 2))
                fs_bstate["rs8"] = rs8
            rs8 = fs_bstate["rs8"]
            # fused multiply + per-channel prefix sum: in0 is the raw wp
            # PSUM tile (fp32, honest 1x), in1 the bf16 tvi; SUB_DIM_DONE
            # resets the running sum at each channel row
            nc.vector._custom_dve(
                sd_op, out=rs8[:, k],
                in0=wp_ps[:, :, :TOT_N], in1=tvis[ti][:])
            if k == okt - 1 or ti == gsz - 1:
                nk = k + 1
                t0 = g0 + ti - k
                last = rs8[:, :nk, :, TOT_N - 1]
                od = score_d.ap()[t0 * P:(t0 + nk) * P, :]
                sck = spool.tile([P, okt, NCH], dt, tag="sck")
                if ext_eng == "act":
                    nc.scalar.copy(sck[:, :nk, :], last)
                elif ext_eng == "dve":
                    nc.vector.tensor_copy(sck[:, :nk, :], last)
                else:
                    nc.gpsimd.tensor_copy(sck[:, :nk, :], last)
                nc.sync.dma_start(
                    od.rearrange("(k p) c -> p k c", k=nk),
                    sck[:, :nk, :])

        def emit_fs_phase3(g0, gsz, ppts, I_g):
            """Emit stage A per tile; stage B trails blag tiles behind in
            every engine queue, so both streams stay interleaved (no
            convoying, no PE head-block on the wv ring)."""
            blag = cfg.get("blag", 3)
            for i in range(gsz):
                tvi = emit_fs_stageA(i, ppts, I_g)
                fs_bq.append((i, g0, gsz, ppts, tvi))
                while len(fs_bq) > blag:
                    it = fs_bq.pop(0)
                    emit_fs_stageB(it[0], it[1], it[2], it[3], {it[0]: it[4]})

        def flush_fs():
            while fs_bq:
                it = fs_bq.pop(0)
                emit_fs_stageB(it[0], it[1], it[2], it[3], {it[0]: it[4]})

        xt_pre = {}

        def preload_x(g0n, gszn):
            lst = []
            for ti0 in range(0, gszn, 2):
                npair = min(2, gszn - ti0)
                t = g0n + ti0
                xt = xpool.tile([P, 4, 2 * P], dt, tag="xt")
                nc.sync.dma_start(xt[:, :, :npair * P],
                                  xT.ap()[:, :, t * P:(t + npair) * P])
                lst.append(xt)
            xt_pre[g0n] = lst

        def fs_load_first(g0, gsz):
            """Group-0 x loads with the constant loads woven in, ordered so
            Vcm (needed by the first vp matmul) and the upper-half const
            replicas (needed by odd pp pairs) land as early as possible.
            Replicas ride the DVE HWDGE queue, not the Pool-engine SWDGE,
            so the Pool engine is free for its stt stream from t=0."""
            lst = []
            for ti0 in range(0, gsz, 2):
                npair = min(2, gsz - ti0)
                t = g0 + ti0
                xt = xpool.tile([P, 4, 2 * P], dt, tag="xt")
                nc.sync.dma_start(xt[:, :, :npair * P],
                                  xT.ap()[:, :, t * P:(t + npair) * P])
                lst.append(xt)
                if ti0 == 0:
                    # keep the serial HWDGE pipe clear for the x pairs: only
                    # Zs/T4T (startup-critical for pp/tb) ride it; V, the
                    # upper-half replicas and Wcm-rep go through the SWDGE,
                    # whose desc-gen runs on the Pool engine while it is
                    # still idle
                    nc.sync.dma_start(Zs_sb[:], Zs.ap())
                    nc.sync.dma_start(T4T_sb[0:PD], T4T.ap())
                    nc.gpsimd.dma_start(T4T_sb[PD:P], T4T.ap())
                    nc.gpsimd.dma_start(Vcm_sb[0:PD], Vcm.ap())
                    nc.gpsimd.dma_start(Vcm_sb[PD:P], Vcm.ap())
                    nc.gpsimd.dma_start(Wcm_sb[PD:P], Wcm.ap())
            nc.sync.dma_start(Wcm_sb[0:PD], Wcm.ap())
            xt_pre[g0] = lst

        def fs_new_prep(g0, gsz):
            pptb = ps_pp.tile([P, 256 + group * t4w], f32, tag="pptb")
            return {"g0": g0, "gsz": gsz, "pptb": pptb,
                    "tb_ps": pptb[:, 256:256 + gsz * t4w],
                    "ppts": [], "pend": [], "ppt4": None, "np_done": 0,
                    "npairs": (gsz + 1) // 2, "xts": xt_pre.pop(g0)}

        def fs_emit_pair(prep):
            """One pp pair: 8 pp matmuls into the shared pptb bank, the
            joint PSUM->SBUF copy every second pair, and the tb matmuls."""
            pi = prep["np_done"]
            prep["np_done"] = pi + 1
            ti0 = 2 * pi
            gsz = prep["gsz"]
            npair = min(2, gsz - ti0)
            pptb, tb_ps = prep["pptb"], prep["tb_ps"]
            xt = prep["xts"][pi]
            pb = PD * (pi % 2)
            pp_ps = pptb[pb:pb + PD, 0:256]
            for k in range(npair):
                for c in range(4):
                    nc.tensor.matmul(
                        pp_ps[:, k * P:(k + 1) * P], Zs_sb[:, c, :],
                        xt[:, c, k * P:(k + 1) * P],
                        start=(c == 0), stop=(c == 3))
            ceng = {"pool": nc.gpsimd.tensor_copy,
                    "dve": nc.vector.tensor_copy}.get(
                        cfg.get("ppt_eng", "act"), nc.scalar.copy)
            if pi % 2 == 0:
                ppt4 = ppool.tile([P, 2, P], dt, tag="ppt")
                prep["ppt4"] = ppt4
                prep["pend"] = [(pb, ti0, npair)]
            else:
                ppt4 = prep["ppt4"]
                prep["pend"].append((pb, ti0, npair))
            if pi % 2 == 1 or ti0 + 2 >= gsz:
                lo = min(q[0] for q in prep["pend"])
                hi = max(q[0] + PD for q in prep["pend"])
                ceng(ppt4[lo:hi].rearrange("p a b -> p (a b)"),
                     pptb[lo:hi, 0:256])
                for (qb, qt, qn) in prep["pend"]:
                    for k in range(qn):
                        ti = qt + k
                        nc.tensor.matmul(
                            tb_ps[:, ti * t4w:(ti + 1) * t4w],
                            ppt4[qb:qb + PD, k, :], T4T_sb[qb:qb + PD],
                            start=True, stop=True)
                prep["pend"] = []
            for k in range(npair):
                prep["ppts"].append(ppt4[pb:pb + PD, k, :])

        def fs_emit_phase2(prep):
            """Sigmoid pairs + level-batched indicator recursion."""
            gsz, tb_ps = prep["gsz"], prep["tb_ps"]
            a_pm = gpool.tile([P, gsz, 16, 2], dt, tag="apm")
            tb4 = tb_ps.rearrange("p (g j s) -> p g j s", j=16, s=2)
            nc.scalar.activation(a_pm[:], tb4, Sigmoid, scale=2.0)
            I_g = gpool.tile([P, gsz, NODES], dt, tag="ig")
            nc.gpsimd.memset(I_g[:, :, 0], 1.0)
            # (the node-31 pad of I_g is never read in fs mode)
            ind_eng = nc.gpsimd if cfg.get("ind_pool", 0) else nc.vector
            ind_eng.tensor_copy(I_g[:, :, 1:3], a_pm[:, :, 0, :])
            for (j0, j1) in ((1, 3), (3, 7), (7, 15)):
                par = I_g[:, :, j0:j1]
                par2 = bass.AP(par.tensor, par.offset, par.ap + [[0, 2]])
                ind_eng.tensor_mul(I_g[:, :, 2 * j0 + 1:2 * j1 + 1], par2,
                                   a_pm[:, :, j0:j1, :])
            return I_g

        def fs_run():
            """fs flow: next-group prep (x prefetch, pp/tb, sigmoid,
            indicators) is interleaved into the stage-A/B tile stream one
            group ahead, so no engine queue ever head-blocks on a group
            seam's cross-engine pp->copy->tb->sigmoid chain."""
            blag = cfg.get("blag", 3)
            fs_load_first(*groups[0])
            prep = fs_new_prep(*groups[0])
            while prep["np_done"] < prep["npairs"]:
                fs_emit_pair(prep)
            ready = (prep["g0"], prep["gsz"], prep["ppts"],
                     fs_emit_phase2(prep))
            if len(groups) > 1:
                preload_x(*groups[1])
            for gi in range(len(groups)):
                g0, gsz, ppts, I_g = ready
                nxt = groups[gi + 1] if gi + 1 < len(groups) else None
                prep = fs_new_prep(*nxt) if nxt else None
                ready = None

                def prep_step():
                    nonlocal ready
                    if prep is None:
                        return
                    if prep["np_done"] < prep["npairs"]:
                        fs_emit_pair(prep)
                    elif ready is None:
                        ready = (prep["g0"], prep["gsz"], prep["ppts"],
                                 fs_emit_phase2(prep))
                        if gi + 2 < len(groups):
                            preload_x(*groups[gi + 2])

                for i in range(gsz):
                    tvi = emit_fs_stageA(i, ppts, I_g)
                    fs_bq.append((i, g0, gsz, ppts, tvi))
                    prep_step()
                    while len(fs_bq) > blag:
                        it = fs_bq.pop(0)
                        emit_fs_stageB(it[0], it[1], it[2], it[3],
                                       {it[0]: it[4]})
                while prep is not None and ready is None:
                    prep_step()
            flush_fs()

        if mode == "fs":
            fs_run()

        for gi, (g0, gsz) in enumerate(groups):
            if mode == "fs":
                break
            # --- phase 1: load x tiles (paired DMAs), ppT, tree logits ---
            if mode in ("sd", "fs") and cfg.get("pptb", 1):
                # tb and the pp pairs co-reside in one PSUM bank: pp pairs
                # in cols 0:256 of partitions 0:64, tb in cols 256:384
                pptb = ps_pp.tile([P, 256 + group * t4w], f32, tag="pptb")
                tb_ps = pptb[:, 256:256 + gsz * t4w]
                pp_region = pptb[0:PD, 0:256]
            else:
                tb_ps = ps_tb.tile([P, gsz * 16], f32, tag="tb")
                pp_region = None
            ppts = []
            for ti0 in range(0, gsz, 2):
                npair = min(2, gsz - ti0)
                t = g0 + ti0
                if g0 in xt_pre:
                    xt = xt_pre[g0][ti0 // 2]
                else:
                    xt = xpool.tile([P, 4, 2 * P], dt, tag="xt")
                    nc.sync.dma_start(xt[:, :, :npair * P],
                                      xT.ap()[:, :, t * P:(t + npair) * P])
                if not zt_loaded:
                    # issued after the first x DMA: the serial HWDGE
                    # descriptor pipe (~625ns each) sits on the startup
                    # critical path
                    nc.sync.dma_start(Zs_sb[:], Zs.ap())
                    nc.sync.dma_start(T4T_sb[0:PD], T4T.ap())
                    if cheight == P:
                        # replica loads ride the gpsimd SWDGE queue so the
                        # serial HWDGE pipe stays clear for x tiles
                        nc.gpsimd.dma_start(T4T_sb[PD:P], T4T.ap())
                    zt_loaded = True
                if mode in ("sd", "fs"):
                    # consecutive pp pairs alternate the partition halves
                    # of the shared PSUM bank (pp spans only 64
                    # partitions), so one full-height [128, 256] copy
                    # drains TWO pairs (four tiles) at the cost of one:
                    # the free-dim size is unchanged and partitions are
                    # parallel on every engine
                    pi = ti0 // 2
                    pb = PD * (pi % 2)
                    if pp_region is not None:
                        pp_ps = pptb[pb:pb + PD, 0:256]
                    else:
                        pp3 = ps_pp.tile([PD, 2, P], f32, tag="pp")
                        pp_ps = pp3[:].rearrange("p a b -> p (a b)")
                    for k in range(npair):
                        for c in range(4):
                            nc.tensor.matmul(
                                pp_ps[:, k * P:(k + 1) * P], Zs_sb[:, c, :],
                                xt[:, c, k * P:(k + 1) * P],
                                start=(c == 0), stop=(c == 3))
                    ppt_eng = cfg.get("ppt_eng", "act")
                    ceng = {"pool": nc.gpsimd.tensor_copy,
                            "dve": nc.vector.tensor_copy}.get(
                                ppt_eng, nc.scalar.copy)
                    if pp_region is not None and cfg.get("jointcp", 1):
                        if pi % 2 == 0:
                            ppt4 = ppool.tile([P, 2, P], dt, tag="ppt")
                            pend = [(pb, ti0, npair)]
                        else:
                            pend.append((pb, ti0, npair))
                        last_pair = ti0 + 2 >= gsz
                        if pi % 2 == 1 or last_pair:
                            # one copy for every pair buffered in pend
                            lo = min(q[0] for q in pend)
                            hi = max(q[0] + PD for q in pend)
                            ceng(ppt4[lo:hi].rearrange("p a b -> p (a b)"),
                                 pptb[lo:hi, 0:256])
                            for (qb, qt, qn) in pend:
                                for k in range(qn):
                                    ti = qt + k
                                    nc.tensor.matmul(
                                        tb_ps[:, ti * t4w:(ti + 1) * t4w],
                                        ppt4[qb:qb + PD, k, :],
                                        T4T_sb[qb:qb + PD],
                                        start=True, stop=True)
                            pend = []
                        for k in range(npair):
                            ppts.append(ppt4[pb:pb + PD, k, :])
                        continue
                    ppt2 = ppool.tile([PD, 2, P], dt, tag="ppt")
                    fpt = ppt2[:].rearrange("p a b -> p (a b)")
                    ceng(fpt[:, :npair * P], pp_ps[:, :npair * P])
                    for k in range(npair):
                        ti = ti0 + k
                        nc.tensor.matmul(
                            tb_ps[:, ti * t4w:(ti + 1) * t4w], ppt2[:, k, :],
                            T4T_sb[0:PD], start=True, stop=True)
                        ppts.append(ppt2[:, k, :])
                    continue
                for k in range(npair):
                    ti = ti0 + k
                    pp_ps = ps_pp.tile([PD, P], f32, tag="pp")
                    for c in range(4):
                        nc.tensor.matmul(
                            pp_ps[:], Zs_sb[:, c, :],
                            xt[:, c, k * P:(k + 1) * P],
                            start=(c == 0), stop=(c == 3))
                    ppt = ppool.tile([PD, P], dt, tag="ppt")
                    nc.scalar.copy(ppt[:], pp_ps[:])
                    nc.tensor.matmul(
                        tb_ps[:, ti * 16:(ti + 1) * 16], ppt[:], T4T_sb[:],
                        start=True, stop=True)
                    ppts.append(ppt)
            if not wv_loaded:
                # emitted after the first tiles so the W/V loads don't
                # delay the startup-critical x DMAs in the queue
                nc.sync.dma_start(Wcm_sb[0:PD], Wcm.ap())
                nc.sync.dma_start(Vcm_sb[0:PD], Vcm.ap())
                if cheight == P:
                    nc.gpsimd.dma_start(Wcm_sb[PD:P], Wcm.ap())
                    nc.gpsimd.dma_start(Vcm_sb[PD:P], Vcm.ap())
                wv_loaded = True

            # --- phase 2: indicators for the whole group ---
            # a_pm[:, g, j, 0] = sigmoid(+2 tb_j), [..., 1] = sigmoid(-2 tb_j)
            a_pm = gpool.tile([P, gsz, 16, 2], dt, tag="apm")
            if mode in ("sd", "fs"):
                # T4T carries [+4T | -4T] interleaved, so one sigmoid
                # produces both (1+tb)/2 and (1-tb)/2 factors
                tb4 = tb_ps.rearrange("p (g j s) -> p g j s", j=16, s=2)
                nc.scalar.activation(a_pm[:], tb4, Sigmoid, scale=2.0)
            else:
                tb3 = tb_ps[:].rearrange("p (g j) -> p g j", j=16)
                nc.scalar.activation(a_pm[:, :, :, 0], tb3, Sigmoid,
                                     scale=2.0)
                nc.scalar.activation(a_pm[:, :, :, 1], tb3, Sigmoid,
                                     scale=-2.0)
            I_g = gpool.tile([P, gsz, NODES], dt, tag="ig")
            nc.gpsimd.memset(I_g[:, :, 0], 1.0)
            nc.gpsimd.memset(I_g[:, :, NODES - 1], 0.0)
            # level-batched recursion: children of level-L parents j0..j1-1
            # are the contiguous nodes 2*j0+1 .. 2*j1, each parent value
            # broadcast over its (+,-) pair via a 0-stride trailing dim
            ind_eng = (nc.gpsimd
                       if (mode in ("sd", "fs") and cfg.get("ind_pool", 0))
                       else nc.vector)
            ind_eng.tensor_copy(I_g[:, :, 1:3], a_pm[:, :, 0, :])
            for (j0, j1) in ((1, 3), (3, 7), (7, 15)):
                par = I_g[:, :, j0:j1]
                par2 = bass.AP(par.tensor, par.offset, par.ap + [[0, 2]])
                ind_eng.tensor_mul(I_g[:, :, 2 * j0 + 1:2 * j1 + 1], par2,
                                   a_pm[:, :, j0:j1, :])

            # --- phase 3 (fs): prefetch next group's x tiles so the next
            # phase-1 pp matmuls never head-block PE at the group seam, then
            # emit stage A with the lagging stage-B stream ---
            if mode == "fs":
                if cfg.get("xpre", 1) and gi + 1 < len(groups):
                    g0n, gszn = groups[gi + 1]
                    if g0n not in xt_pre:
                        preload_x(g0n, gszn)
                emit_fs_phase3(g0, gsz, ppts, I_g)
                continue

            # --- phase 3 (sd): software-pipelined one group behind, so
            # each engine's queue interleaves next-group prep with this
            # group's heavy work instead of convoying ---
            if mode == "sd":
                sd_pending.append((g0, gsz, ppts, I_g))
                if len(sd_pending) > 1:
                    emit_sd_phase3(*sd_pending.pop(0))
                continue

            # --- phase 3 (paired): two batch tiles per DVE op ---
            if mode == "scan2":
                for ti in range(0, gsz, 2):
                    t = g0 + ti
                    wp2 = ps_w2.tile([P, 2, NCH, NODES], f32, tag="wp2")
                    fw2 = wp2[:].rearrange("p k c n -> p (k c n)")
                    tv2 = vpool.tile([P, 2, NCH, NODES], dt, tag="tv2")
                    for k in (0, 1):
                        ppt = ppts[ti + k]
                        o = k * FREE
                        nc.tensor.matmul(fw2[:, o:o + 512], ppt[:],
                                         Wcm_sb[:, 0:512], start=True, stop=True)
                        nc.tensor.matmul(fw2[:, o + 512:o + 1024], ppt[:],
                                         Wcm_sb[:, 512:1024], start=True, stop=True)
                        vp_ps = ps_wv.tile([P, NCH, NODES], f32, tag="vp")
                        fv = _flat(vp_ps[:])
                        nc.tensor.matmul(fv[:, 0:512], ppt[:],
                                         Vcm_sb[:, 0:512], start=True, stop=True)
                        nc.tensor.matmul(fv[:, 512:1024], ppt[:],
                                         Vcm_sb[:, 512:1024], start=True, stop=True)
                        nc.scalar.activation(
                            tv2[:, k].rearrange("p c n -> p (c n)"), fv, Tanh)
                    ib = I_g[:, ti:ti + 2, :]
                    ib4 = bass.AP(ib.tensor, ib.offset,
                                  [ib.ap[0], ib.ap[1], [0, NCH], ib.ap[2]])
                    tvi2 = vpool.tile([P, 2, NCH, NODES], dt, tag="tvi2")
                    nc.vector.tensor_mul(tvi2[:], tv2[:], ib4)
                    rs2 = vpool.tile([P, 2, NCH, NODES], f32, tag="rs2")
                    nc.vector._custom_dve(
                        scan_op, out=rs2[:].rearrange("p k c n -> p (k c n)"),
                        in0=fw2,
                        in1=tvi2[:].rearrange("p k c n -> p (k c n)"))
                    sc2 = spool.tile([P, 2, NCH], f32, tag="sc2")
                    nc.vector.tensor_copy(sc2[:], rs2[:, :, :, NODES - 1])
                    od = score_d.ap()[t * P:(t + 2) * P, :]
                    nc.sync.dma_start(
                        od.rearrange("(k p) c -> p k c", k=2), sc2[:])
                continue

            # --- phase 3: wp/vp matmuls, elementwise, reduce, store ---
            sc2 = None
            for ti in range(gsz):
                t = g0 + ti
                ppt = ppts[ti]
                wp_ps = ps_wp.tile([P, NCH, NODES], f32, tag="wp")
                vp_ps = ps_wv.tile([P, NCH, NODES], f32, tag="vp")
                fw = _flat(wp_ps[:])
                fv = _flat(vp_ps[:])
                nc.tensor.matmul(fw[:, 0:512], ppt[:], Wcm_sb[:, 0:512],
                                 start=True, stop=True)
                nc.tensor.matmul(fw[:, 512:1024], ppt[:], Wcm_sb[:, 512:1024],
                                 start=True, stop=True)
                nc.tensor.matmul(fv[:, 0:512], ppt[:], Vcm_sb[:, 0:512],
                                 start=True, stop=True)
                nc.tensor.matmul(fv[:, 512:1024], ppt[:], Vcm_sb[:, 512:1024],
                                 start=True, stop=True)
                tv = vpool.tile([P, NCH, NODES], dt, tag="tv")
                nc.scalar.activation(_flat(tv[:]), fv, Tanh)
                # I broadcast over the channel (middle) dim via a 0-stride AP
                ib = I_g[:, ti, :]
                ib3 = bass.AP(ib.tensor, ib.offset,
                              [ib.ap[0], [0, NCH], ib.ap[1]])
                if mode == "scan":
                    tvi = vpool.tile([P, NCH, NODES], dt, tag="tvi")
                    nc.vector.tensor_mul(tvi[:], tv[:], ib3)
                    # the scan skips the zero pad node (31 real nodes per
                    # channel block): custom-DVE rate is 1 elem/cycle
                    # regardless of AP shape, so odd counts cost nothing
                    k = ti % 2
                    if k == 0:
                        sc2 = spool.tile([P, 2, NCH], f32, tag="sc2")
                    rs = vpool.tile([P, NCH, TOT_N], f32, tag="rs")
                    nc.vector._custom_dve(
                        scan_op, out=rs[:], in0=wp_ps[:, :, :TOT_N],
                        in1=tvi[:, :, :TOT_N])
                    nc.vector.tensor_copy(sc2[:, k, :],
                                          rs[:, :, TOT_N - 1])
                    if k == 1 or ti == gsz - 1:
                        npair = k + 1
                        t0 = g0 + ti - k
                        od = score_d.ap()[t0 * P:(t0 + npair) * P, :]
                        nc.sync.dma_start(
                            od.rearrange("(k p) c -> p k c", k=npair),
                            sc2[:, :npair, :])
                    continue
                sc = spool.tile([P, NCH], f32, tag="sc")
                if True:
                    prod1 = vpool.tile([P, NCH, NODES], dt, tag="p1")
                    if fp32:
                        nc.vector.tensor_mul(_flat(prod1[:]), fw, _flat(tv[:]))
                    else:
                        # cross wp PSUM->SBUF on ScalarE so the DVE multiplies
                        # run in the 2x bf16 mode (SBUF-only operands)
                        wp_sb = vpool.tile([P, NCH, NODES], dt, tag="wpsb")
                        nc.scalar.copy(_flat(wp_sb[:]), fw)
                        nc.vector.tensor_mul(_flat(prod1[:]), _flat(wp_sb[:]),
                                             _flat(tv[:]))
                    prod2 = vpool.tile([P, NCH, NODES], dt, tag="p2")
                    nc.vector.tensor_mul(prod2[:], prod1[:], ib3)
                    nc.vector.tensor_reduce(
                        sc[:], prod2[:], axis=mybir.AxisListType.X,
                        op=mybir.AluOpType.add)
                nc.sync.dma_start(score_d.ap()[t * P:(t + 1) * P, :], sc[:])

        if mode == "fs":
            flush_fs()
        while sd_pending:
            emit_sd_phase3(*sd_pending.pop(0))

    nc.compile()
    return nc


def _prep_inputs(x, Z, T, V, W, fp32=True, pm=False):
    """Host-side: shard + relayout inputs for the per-core program."""
    import ml_dtypes

    dt = np.float32 if fp32 else ml_dtypes.bfloat16
    xp = np.zeros((N_CORES * B_CORE, NF), np.float32)
    xp[:BATCH] = x
    xp = xp.reshape(N_CORES, B_CORE, NF)

    Zs = np.ascontiguousarray(
        (np.asarray(Z, np.float32) / PD).reshape(4, P, PD).transpose(1, 0, 2)
    ).astype(dt)
    if pm:
        T4T = np.zeros((PD, 16, 2), np.float32)
        T4T[:, :INT_N, 0] = (SIG * np.asarray(T, np.float32)).T
        T4T[:, :INT_N, 1] = -(SIG * np.asarray(T, np.float32)).T
        T4T = T4T.reshape(PD, 32).astype(dt)
    else:
        T4T = np.zeros((PD, 16), np.float32)
        T4T[:, :INT_N] = (SIG * np.asarray(T, np.float32)).T
        T4T = T4T.astype(dt)
    Wcm = np.zeros((PD, NCH, NODES), np.float32)
    Wcm[:, :, :TOT_N] = np.asarray(W, np.float32).transpose(1, 2, 0)
    Wcm = Wcm.reshape(PD, FREE).astype(dt)
    Vcm = np.zeros((PD, NCH, NODES), np.float32)
    Vcm[:, :, :TOT_N] = (SIG * np.asarray(V, np.float32)).transpose(1, 2, 0)
    Vcm = Vcm.reshape(PD, FREE).astype(dt)

    in_maps = []
    for c in range(N_CORES):
        xT = np.ascontiguousarray(
            xp[c].T.reshape(4, P, B_CORE).transpose(1, 0, 2)).astype(dt)
        in_maps.append(
            {"xT": xT, "Zs": Zs, "T4T": T4T, "Wcm": Wcm, "Vcm": Vcm})
    return in_maps


def kernel(x, Z, T, V, W):
    import os

    from concourse.bass_utils import run_bass_kernel_spmd

    mode = os.environ.get("BONSAI_MODE", "fs")
    try:
        key = ("nc", TILES_FULL, mode)
        if key not in _CACHE:
            _CACHE[key] = _build(TILES_FULL, 8, mode=mode)
        nc = _CACHE[key]
        in_maps = _prep_inputs(x, Z, T, V, W, fp32=(mode == "fp32"),
                               pm=(mode in ("sd", "fs")))
        res = run_bass_kernel_spmd(
            nc, in_maps, list(range(N_CORES)),
            trace=bool(os.environ.get("BONSAI_TRACE")))
    except Exception:
        if mode not in ("scan", "sd", "fs"):
            raise
        # fall back to the stock-op pipeline if the custom-DVE path is
        # unavailable in this environment
        mode = "bf16"
        key = ("nc", TILES_FULL, mode)
        if key not in _CACHE:
            _CACHE[key] = _build(TILES_FULL, 8, mode=mode)
        nc = _CACHE[key]
        in_maps = _prep_inputs(x, Z, T, V, W, fp32=False)
        res = run_bass_kernel_spmd(
            nc, in_maps, list(range(N_CORES)),
            trace=bool(os.environ.get("BONSAI_TRACE")))
    _CACHE["last_result"] = res
    out = np.concatenate(
        [np.asarray(r["score"], np.float32) for r in res.results], axis=0)
    if mode in ("scan", "scan2"):
        raw = out.copy()
        out[:, 1:] = raw[:, 1:] - raw[:, :-1]
        if mode == "scan2":
            # odd tiles continue the even partner tile's running sum
            v_out = out.reshape(-1, 2, P, NCH)
            v_raw = raw.reshape(-1, 2, P, NCH)
            v_out[:, 1, :, 0] = v_raw[:, 1, :, 0] - v_raw[:, 0, :, NCH - 1]
    return np.ascontiguousarray(out[:BATCH]).astype(np.float32)

